# revision 1
# baseline (speedup 1.0000x reference)
"""Trainium2 Bass kernel for the colorization loss.

Math (restructured from the reference, validated to rel-err ~1e-6):
  For each pixel with chroma (a, b):
    m(q)  = 2*a*gx_q + 2*b*gy_q - (gx_q^2 + gy_q^2)   # = (a^2+b^2) - d^2(q)
    top-5 largest m (== 5 smallest distances, ascending), m_0 >= ... >= m_4
    e_k   = exp((m_k - m_0)/50)                        # per-pixel offset cancels
    p_k   = e_k / sum_j e_j                            # == softencode weights
    lse   = log(sum_q exp(Zbar_q))                     # log-softmax denominator
    loss  = mean over pixels of  sum_k reb_k * p_k * (lse - Zbar_k)
          = mean of (lse * sum_k(reb_k e_k) - sum_k(reb_k e_k Zbar_k)) / sum_k e_k

Per-core layout (data-parallel over batch, 2 images / core):
  zbar [32768, 313] rows = pixels; tiles of 128 consecutive pixels.
  Per tile: PE matmul (K=3) -> m in PSUM; DVE max8 -> top-8; ACT exp+accum
  -> sum_exp; gpsimd copies Zbar[:, :5]; small batched epilogue every 64 tiles.
"""

import numpy as np

import concourse.bass as bass
import concourse.tile as tile
from concourse import mybir
from concourse.bass_utils import run_bass_kernel_spmd

# Problem shape (hardcoded: nn_ColorizationLoss, B,H,W,Q = 16,128,128,313)
B, H, W, Q = 16, 128, 128, 313
NCORES = 8
B_PER = B // NCORES            # 2 images per core
PIX = B_PER * H * W            # 32768 pixels per core
P = 128                        # SBUF partitions / pixels per tile
NT = PIX // P                  # 256 tiles per core
GT = 16                        # tiles per zbar DMA group
NG = NT // GT                  # 16 groups
TB = 32                        # tiles per epilogue batch
NB = NT // TB                  # 8 batches
TOPK = 5
INV50 = 1.0 / 50.0             # 1/(2*sigma^2), sigma=5
HPIX = PIX // 2                # pixels per abx segment (2 partition quadrants)
ABXC = HPIX + Q                # abx columns (pixel data + gamut rhs block)

f32 = mybir.dt.float32
AF = mybir.ActivationFunctionType
AX = mybir.AxisListType

_NC = None


def _build_nc():
    nc = bass.Bass()
    zbar_d = nc.dram_tensor("zbar", [PIX, Q], f32, kind="ExternalInput")
    # abx packs [a; b; 1] per pixel AND the gamut rhs matrix [2gx; 2gy;
    # -|g|^2] in ONE tensor/DMA (PE LDWEIGHTS can carry only one sem wait).
    # Two segments on partition rows 0-2 and 64-66 (matmul base-partition
    # must be 0/32/64) halve the per-partition SBUF footprint; the gamut
    # block is replicated in both segments at cols HPIX:HPIX+Q.
    abx_d = nc.dram_tensor("abx", [6, ABXC], f32, kind="ExternalInput")
    reb_d = nc.dram_tensor("rebt", [P, TB * TOPK], f32, kind="ExternalInput")
    out_d = nc.dram_tensor("acc", [P, 1], f32, kind="ExternalOutput")

    # zbar viewed as [group g][partition p][tile-in-group j][q]
    zbar_g = zbar_d[:, :].rearrange("(g j p) q -> g p j q", j=GT, p=P)
    # zbar viewed per epilogue batch for the channels-0:5 side load
    zbar_b = zbar_d[:, :].rearrange("(b t p) q -> b p t q", t=TB, p=P)

    with tile.TileContext(nc) as tc:
        with (
            tc.tile_pool(name="singles", bufs=1) as singles,
            tc.tile_pool(name="zg", bufs=4) as zgp,
            tc.tile_pool(name="es", bufs=3) as esp,
            tc.tile_pool(name="epi", bufs=2) as epi,
            tc.tile_pool(name="ps", bufs=4, space="PSUM") as psp,
        ):
            abx_sb = singles.tile([67, ABXC], f32)
            nc.sync.dma_start(out=abx_sb[0:3, :], in_=abx_d[0:3, :])
            nc.sync.dma_start(out=abx_sb[64:67, :], in_=abx_d[3:6, :])
            # stage rebalance through a DVE copy so epilogue DVE reads are
            # same-engine (each compute instr can carry only one sem wait)
            reb_st = singles.tile([P, TB, TOPK], f32)
            nc.sync.dma_start(out=reb_st, in_=reb_d[:, :].rearrange("p (t k) -> p t k", k=TOPK))
            reb_sb = singles.tile([P, TB, TOPK], f32)
            nc.vector.tensor_copy(reb_sb, reb_st)
            acc = singles.tile([P, 1], f32)
            nc.vector.memset(acc, 0.0)

            # Full-size result buffers (one column range per tile, never
            # recycled) — eliminates all WAR/WAW slot hazards, whose sem
            # waits collide with the 1-wait-per-instruction hardware limit.
            Sf = singles.tile([P, NT], f32)          # sum_q exp(zbar)
            Wf = singles.tile([P, NT, 8], f32)       # top-8 of m
            Xf = singles.tile([P, NT, TOPK], f32)    # m_k - m_0
            Zf = singles.tile([P, NT, TOPK], f32)    # zbar[:, :5]
            ppsum_prev = None

            # channels 0:5 of zbar side-loaded straight from DRAM (so the
            # big zbar tiles have a single reader engine); Zf is full-size,
            # so all batches' loads can start immediately
            for bi in range(NB):
                nc.sync.dma_start(
                    out=Zf[:, bi * TB:(bi + 1) * TB],
                    in_=zbar_b[bi][:, :, 0:TOPK],
                )

            # zbar group triggers on the ACT ring, issued two groups ahead:
            # with bufs=4 the claimed slot's readers finished two whole
            # groups before the emission point, so the WAR is covered by
            # sequencer program order and each trigger carries at most the
            # (legal) single epoch wait.
            zg_pending = {}

            def issue_zg(g):
                zgt = zgp.tile([P, GT, Q], f32, tag="zg", name=f"zg{g}")
                nc.scalar.dma_start(out=zgt, in_=zbar_g[g])
                zg_pending[g] = zgt

            issue_zg(0)
            issue_zg(1)

            for bi in range(NB):
                sl = slice(bi * TB, (bi + 1) * TB)
                S, Wt, Xt, Z5 = Sf[:, sl], Wf[:, sl], Xf[:, sl], Zf[:, sl]

                for gi in range(TB // GT):
                    g = bi * (TB // GT) + gi
                    if g + 2 < NG:
                        issue_zg(g + 2)
                    zg = zg_pending.pop(g)
                    for j in range(GT):
                        ti = gi * GT + j                    # tile within batch
                        t = bi * TB + ti                    # global tile
                        so = 64 * (t // (NT // 2))          # segment row base
                        col = (t % (NT // 2)) * P
                        # the first matmul of segment 2 waits on that
                        # segment's abx DMA; a fresh psum tag keeps its
                        # slot-recycle DVE wait out of the instruction
                        pstag = "psb" if t == NT // 2 else "ps"
                        ps = psp.tile([P, Q], f32, tag=pstag)
                        nc.tensor.matmul(
                            ps,
                            abx_sb[so:so + 3, col:col + P],
                            abx_sb[so:so + 3, HPIX:HPIX + Q],
                            start=True,
                            stop=True,
                        )
                        # group-boundary exp (j==0) uses its own scratch tag:
                        # its WAW dep is then ancient, so it carries only the
                        # zbar-DMA wait (compute instrs allow a single wait)
                        es = esp.tile([P, Q], f32, tag="esb" if j == 0 else "es")
                        nc.scalar.activation(
                            out=es, in_=zg[:, j, :], func=AF.Exp,
                            accum_out=S[:, ti:ti + 1],
                        )
                        nc.vector.max(out=Wt[:, ti, :], in_=ps)
                        nc.vector.tensor_scalar_sub(
                            Xt[:, ti, :], Wt[:, ti, 0:TOPK], Wt[:, ti, 0:1]
                        )

                # ---- batched epilogue over TB tiles ----
                # Ordered so each DVE instruction has exactly one uncovered
                # dependency (1 sem wait per instruction hardware limit):
                # cross-engine inputs enter the chain only via instructions
                # whose other operands are already covered by earlier waits.
                if ppsum_prev is not None:
                    # advances DVE's observed self-clock past the whole
                    # previous epilogue, eliding pooled-buffer WAW waits
                    nc.vector.memset(ppsum_prev, 0.0)
                E = epi.tile([P, TB, TOPK], f32, tag="E")
                # ACT bump: one DVE wait covers both the Xt chain (subs) and
                # the E-slot release, so the exp below carries only its
                # (legal) self wait
                nc.scalar.activation(out=E[:, 0:1, 0:1], in_=Xt[:, TB - 1:TB, 0:1],
                                     func=AF.Copy)
                nc.scalar.activation(out=E, in_=Xt, func=AF.Exp, scale=INV50)
                U = epi.tile([P, TB, TOPK], f32, tag="U")
                nc.vector.tensor_mul(U, E, reb_sb)          # {ACT}
                s2 = epi.tile([P, TB + 1], f32, tag="s2")
                nc.vector.reduce_sum(s2[:, 0:TB], U, axis=AX.X)  # {DVE>=U}
                UZ = epi.tile([P, TB, TOPK], f32, tag="UZ")
                nc.vector.tensor_mul(UZ, U, Z5)             # {DMA-Z5}
                s1 = epi.tile([P, TB], f32, tag="s1")
                nc.vector.reduce_sum(s1, UZ, axis=AX.X)     # {DVE>=UZ}
                sw = epi.tile([P, TB], f32, tag="sw")
                nc.vector.reduce_sum(sw, E, axis=AX.X)      # covered
                lse = epi.tile([P, TB], f32, tag="lse")
                nc.scalar.activation(out=lse, in_=S, func=AF.Ln)
                # DVE bump: absorbs the s2 chain dep (the scheduler may run
                # this before s1's reduce, leaving s2's tick uncovered)
                nc.vector.tensor_copy(s2[:, TB:TB + 1], s2[:, 0:1])
                t1 = epi.tile([P, TB], f32, tag="t1")
                nc.vector.tensor_mul(t1, lse, s2[:, 0:TB])  # {ACT>=Ln}
                nc.vector.tensor_sub(t1, t1, s1)            # {DVE}
                r = epi.tile([P, TB], f32, tag="r")
                nc.vector.reciprocal(r, sw)
                nc.vector.tensor_mul(t1, t1, r)
                ppsum = epi.tile([P, 1], f32, tag="ppsum")
                nc.vector.reduce_sum(ppsum, t1, axis=AX.X)
                nc.vector.tensor_add(acc, acc, ppsum)
                ppsum_prev = ppsum

            # SWDGE (gpsimd) ring: fresh sem pool, so no epoch wait joins
            # the {DVE} data wait on this final transfer
            nc.gpsimd.dma_start(out=out_d[:, :], in_=acc)

    # The kernel-tail drain waits on every used proc (11+ sems) which
    # exceeds the instruction's sync-wait capacity. Every instruction in
    # this kernel is transitively upstream of the final out DMA (acc is the
    # sink), so waiting for that DMA's SWDGE sem alone is sufficient.
    for blk in nc.m.functions[0].blocks:
        for inst in blk.instructions:
            si = getattr(inst, "sync_info", None)
            if si is None or type(inst).__name__ != "InstDrain":
                continue
            ge = [w for w in si.on_wait if w.wait_mode == "sem-ge-imm"]
            if len(ge) >= 2:
                sw = [w for w in ge if "DMASW" in w.ant_name]
                assert sw, f"tail drain has no SWDGE wait: {ge}"
                si.on_wait = sw[:1]
    return nc


def _get_nc():
    global _NC
    if _NC is None:
        _NC = _build_nc()
    return _NC


def make_in_maps(Zbar, Y, rebalance, gamut):
    Zbar = np.asarray(Zbar, dtype=np.float32)
    Y = np.asarray(Y, dtype=np.float32)
    rebalance = np.asarray(rebalance, dtype=np.float32)
    gamut = np.asarray(gamut, dtype=np.float32)

    gx, gy = gamut[:, 0], gamut[:, 1]
    rhs = np.stack([2.0 * gx, 2.0 * gy, -(gx * gx + gy * gy)]).astype(np.float32)
    rebt = np.ascontiguousarray(
        np.broadcast_to(np.tile(rebalance[:TOPK], TB)[None, :], (P, TB * TOPK))
    ).astype(np.float32)

    in_maps = []
    for c in range(NCORES):
        sl = slice(c * B_PER, (c + 1) * B_PER)
        zb = np.ascontiguousarray(Zbar[sl].reshape(PIX, Q))
        a = Y[sl, 1].reshape(PIX)
        b = Y[sl, 2].reshape(PIX)
        abx = np.zeros((6, ABXC), np.float32)
        for s in range(2):
            px = slice(s * HPIX, (s + 1) * HPIX)
            abx[3 * s + 0, :HPIX] = a[px]
            abx[3 * s + 1, :HPIX] = b[px]
            abx[3 * s + 2, :HPIX] = 1.0
            abx[3 * s:3 * s + 3, HPIX:] = rhs
        in_maps.append({"zbar": zb, "abx": abx, "rebt": rebt})
    return in_maps


def kernel(Zbar, Y, rebalance, gamut):
    in_maps = make_in_maps(Zbar, Y, rebalance, gamut)
    res = run_bass_kernel_spmd(_get_nc(), in_maps, list(range(NCORES)))
    total = sum(float(r["acc"].sum(dtype=np.float64)) for r in res.results)
    return np.float32(total / (B * H * W))



# revision 17
# speedup vs baseline: 1.4232x; 1.4232x over previous
"""Trainium2 Bass kernel for the colorization loss (v2).

Math (restructured from the reference):
  For each pixel with chroma (a, b):
    m(q)  = 2*a*gx_q + 2*b*gy_q - (gx_q^2 + gy_q^2)   # = (a^2+b^2) - d^2(q)
    top-5 largest m (== 5 smallest distances, ascending), m_0 >= ... >= m_4
    e_k   = exp((m_k - m_0)/50)                        # per-pixel offset cancels
    lse   = log(sum_q exp(Zbar_q))                     # log-softmax denominator
    loss  = mean of (lse * sum_k(reb_k e_k) - sum_k(reb_k e_k Zbar_k)) / sum_k e_k

Key engine assignments (per 128-pixel tile; 256 tiles/core):
  PE   : m matmul (K=3, fp32), 4-way row-tiled via tile_position so
         consecutive tiles run CONCURRENTLY in different 32-row groups.
  DVE  : max8 top-8 of m (the unavoidable 125us), segmented exp-sum
         reduce for the first NGD groups, slim batched epilogue.
  ACT  : exp of zbar. Per-tile exp+accum for most groups (sum on ACT);
         one big group-exp for the first NGD groups (sum on DVE).
         NGD balances ACT vs DVE busy time.
  GPSIMD: Zbar[:, :5] extraction by SBUF copy (replaces descriptor-heavy
         strided DMA), epilogue elementwise muls.
  DMA  : zbar shipped as bf16 (host converts) -> 20.5 MB/core instead of 41.
"""

import numpy as np
import ml_dtypes

import concourse.bass as bass
import concourse.tile as tile
from concourse import mybir
from concourse.bass_utils import run_bass_kernel_spmd

# Problem shape (hardcoded: nn_ColorizationLoss, B,H,W,Q = 16,128,128,313)
B, H, W, Q = 16, 128, 128, 313
NCORES = 8
B_PER = B // NCORES            # 2 images per core
PIX = B_PER * H * W            # 32768 pixels per core
P = 128                        # SBUF partitions / pixels per tile
NT = PIX // P                  # 256 tiles per core
GT = 16                        # tiles per zbar DMA group
NG = NT // GT                  # 16 groups
TB = 64                        # tiles per epilogue batch
NB = NT // TB                  # 4 batches
TOPK = 5
INV50 = 1.0 / 50.0             # 1/(2*sigma^2), sigma=5
NSEG = 4                       # PE row-groups used for the m matmul
PIXSEG = PIX // NSEG           # pixels per abx segment
ABXC = PIXSEG + Q              # abx columns (pixel block + gamut rhs block)
DVE_GROUPS = (1, 5, 9)         # groups whose exp-sum runs on DVE (rest: ACT accum)

f32 = mybir.dt.float32
bf16 = mybir.dt.bfloat16
AF = mybir.ActivationFunctionType
AX = mybir.AxisListType

_NC = None


def _build_nc():
    nc = bass.Bass()
    zbar_d = nc.dram_tensor("zbar", [PIX, Q], bf16, kind="ExternalInput")
    # abx packs, per row-group segment s (3 rows each): [a; b; 1] for the
    # pixels of tiles t with t % 4 == s, plus the gamut rhs matrix
    # [2gx; 2gy; -|g|^2] at cols PIXSEG:PIXSEG+Q.
    abx_d = nc.dram_tensor("abx", [3 * NSEG, ABXC], f32, kind="ExternalInput")
    # zbar[:, :5] pre-transposed on host to [P, NT*5] so one DMA with one
    # contiguous run per partition fills Zf (no strided descriptor spam)
    zq_d = nc.dram_tensor("zq", [P, NT * TOPK], f32, kind="ExternalInput")
    reb_d = nc.dram_tensor("rebt", [P, TB * TOPK], f32, kind="ExternalInput")
    out_d = nc.dram_tensor("acc", [P, 1], f32, kind="ExternalOutput")

    # zbar viewed as [group g][partition p][tile-in-group j][q]
    zbar_g = zbar_d[:, :].rearrange("(g j p) q -> g p j q", j=GT, p=P)

    with tile.TileContext(nc) as tc:
        with (
            tc.tile_pool(name="singles", bufs=1) as singles,
            tc.tile_pool(name="zg", bufs=5) as zgp,
            tc.tile_pool(name="esg", bufs=3) as esgp,
            tc.tile_pool(name="est", bufs=2) as estp,
            tc.tile_pool(name="ps", bufs=8, space="PSUM") as psp,
        ):
            abx_sb = singles.tile([32 * (NSEG - 1) + 3, ABXC], f32)
            for s in range(NSEG):
                nc.sync.dma_start(
                    out=abx_sb[32 * s:32 * s + 3, :], in_=abx_d[3 * s:3 * s + 3, :]
                )
            # stage rebalance through an ACT copy: the gpsimd U-mul then waits
            # on the single Activation sem, which covers BOTH its E input and
            # reb (one legal sem wait instead of two)
            reb_st = singles.tile([P, TB, TOPK], f32)
            nc.sync.dma_start(
                out=reb_st, in_=reb_d[:, :].rearrange("p (t k) -> p t k", k=TOPK)
            )
            reb_sb = singles.tile([P, TB, TOPK], f32)
            nc.scalar.copy(reb_sb, reb_st)
            acc = singles.tile([P, 1], f32)
            nc.vector.memset(acc, 0.0)

            # Full-size buffers (one column range per tile/batch, never
            # recycled) — no WAR/WAW slot hazards, so every cross-engine
            # dependency costs at most the single legal sem wait.
            Sf = singles.tile([P, NT], f32)          # sum_q exp(zbar)
            Wf = singles.tile([P, NT, 8], f32)       # top-8 of m
            Zf = singles.tile([P, NT, TOPK], f32)    # zbar[:, :5]
            nc.sync.dma_start(
                out=Zf, in_=zq_d[:, :].rearrange("p (t k) -> p t k", k=TOPK)
            )
            # gpsimd warmup touch of Zf: absorbs the Zf-DMA wait into Pool
            # program order so the epilogue UZ muls carry no DMA wait
            zf_tch = singles.tile([P, 1], f32)
            nc.gpsimd.tensor_copy(zf_tch, Zf[:, 0, 0:1])
            # bump scratch (see epilogue)
            bmpa = singles.tile([P, NB], f32)
            bmpv = singles.tile([P, NB], f32)
            Xf = singles.tile([P, NT, TOPK], f32)    # m_k - m_0
            Ef = singles.tile([P, NT, TOPK], f32)    # exp(X/50)
            Uf = singles.tile([P, NT, TOPK], f32)    # E * reb
            UZf = singles.tile([P, NT, TOPK], f32)   # U * zbar[:, :5]
            s2f = singles.tile([P, NT], f32)
            s1f = singles.tile([P, NT], f32)
            swf = singles.tile([P, NT], f32)
            lsef = singles.tile([P, NT], f32)
            t1f = singles.tile([P, NT], f32)
            rf = singles.tile([P, NT], f32)
            ppf = singles.tile([P, NB], f32)

            zg_pending = {}

            # zbar group triggers on the ACT ring: ACT is a reader of zg, so
            # the slot's ACT WAR is covered by sequencer program order and the
            # trigger carries only the gpsimd (Zf copy) wait.
            def issue_zg(g):
                zgt = zgp.tile([P, GT, Q], bf16, tag="zg", name=f"zg{g}")
                nc.scalar.dma_start(out=zgt, in_=zbar_g[g])
                zg_pending[g] = zgt

            issue_zg(0)
            issue_zg(1)

            for g in range(NG):
                if g + 2 < NG:
                    issue_zg(g + 2)
                zg = zg_pending.pop(g)

                for j in range(GT):
                    t = g * GT + j
                    s = t % NSEG                     # PE row-group
                    blk = t // NSEG                  # column block in segment
                    ps = psp.tile([P, Q], f32, tag="ps")
                    nc.tensor.matmul(
                        ps,
                        abx_sb[32 * s:32 * s + 3, blk * P:(blk + 1) * P],
                        abx_sb[32 * s:32 * s + 3, PIXSEG:PIXSEG + Q],
                        start=True,
                        stop=True,
                        tile_position=(32 * s, 0),
                    )
                    nc.vector.max(out=Wf[:, t, :], in_=ps)

                if g in DVE_GROUPS:
                    # group-sized exp on ACT; per-tile sums on DVE
                    esg = esgp.tile([P, GT, Q], bf16, tag="esg")
                    nc.scalar.activation(out=esg, in_=zg[:, :, :], func=AF.Exp)
                    nc.vector.reduce_sum(
                        Sf[:, g * GT:(g + 1) * GT], esg, axis=AX.X
                    )
                else:
                    # per-tile exp with ACT-side accumulation. The group
                    # boundary (j == 0) uses its own scratch tag: its WAW dep
                    # is then ancient, so it carries only the zbar-DMA wait
                    # (compute instructions allow a single sem wait).
                    for j in range(GT):
                        t = g * GT + j
                        es = estp.tile([P, Q], bf16, tag="esb" if j == 0 else "est")
                        nc.scalar.activation(
                            out=es, in_=zg[:, j, :], func=AF.Exp,
                            accum_out=Sf[:, t:t + 1],
                        )

                # ---- batched epilogue every TB tiles ----
                if (g + 1) % (TB // GT) == 0:
                    bi = (g + 1) // (TB // GT) - 1
                    sl = slice(bi * TB, (bi + 1) * TB)
                    Xt = Xf[:, sl]
                    nc.vector.tensor_sub(
                        Xt, Wf[:, sl, 0:TOPK],
                        Wf[:, sl, 0:1].broadcast_to([P, TB, TOPK]),
                    )
                    E = Ef[:, sl]
                    nc.scalar.activation(out=E, in_=Xt, func=AF.Exp, scale=INV50)
                    U = Uf[:, sl]
                    nc.gpsimd.tensor_mul(U, E, reb_sb)
                    UZ = UZf[:, sl]
                    nc.gpsimd.tensor_mul(UZ, U, Zf[:, sl, :])
                    s2 = s2f[:, sl]
                    nc.vector.reduce_sum(s2, U, axis=AX.X)
                    s1 = s1f[:, sl]
                    nc.vector.reduce_sum(s1, UZ, axis=AX.X)
                    sw = swf[:, sl]
                    nc.vector.reduce_sum(sw, E, axis=AX.X)
                    # ACT bump: absorb the DVE dep (this batch's DVE-group
                    # reduce into Sf) so lse itself carries one wait
                    gd = [x for x in DVE_GROUPS if bi * (TB // GT) <= x < (bi + 1) * (TB // GT)]
                    if gd:
                        nc.scalar.copy(bmpa[:, bi:bi + 1], Sf[:, gd[0] * GT:gd[0] * GT + 1])
                    lse = lsef[:, sl]
                    nc.scalar.activation(out=lse, in_=Sf[:, sl], func=AF.Ln)
                    # DVE bump: absorb the ACT lse dep so the t1 chain stays
                    # on DVE program order
                    nc.vector.tensor_copy(bmpv[:, bi:bi + 1], lse[:, 0:1])
                    t1 = t1f[:, sl]
                    nc.vector.tensor_mul(t1, lse, s2)
                    nc.vector.tensor_sub(t1, t1, s1)
                    r = rf[:, sl]
                    nc.vector.reciprocal(r, sw)
                    nc.vector.tensor_mul(t1, t1, r)
                    nc.vector.reduce_sum(ppf[:, bi:bi + 1], t1, axis=AX.X)
                    nc.vector.tensor_add(acc, acc, ppf[:, bi:bi + 1])

            # SWDGE (gpsimd) ring: fresh sem pool for the final transfer
            nc.gpsimd.dma_start(out=out_d[:, :], in_=acc)

    # The kernel-tail drain waits on every used proc which can exceed the
    # instruction's sync-wait capacity. Every instruction is transitively
    # upstream of the final out DMA (acc is the sink), so waiting for that
    # DMA's SWDGE sem alone is sufficient.
    for blk in nc.m.functions[0].blocks:
        for inst in blk.instructions:
            si = getattr(inst, "sync_info", None)
            if si is None or type(inst).__name__ != "InstDrain":
                continue
            ge = [w for w in si.on_wait if w.wait_mode == "sem-ge-imm"]
            if len(ge) >= 2:
                sw_ = [w for w in ge if "DMASW" in w.ant_name]
                assert sw_, f"tail drain has no SWDGE wait: {ge}"
                si.on_wait = sw_[:1]
    return nc


def _get_nc():
    global _NC
    if _NC is None:
        _NC = _build_nc()
    return _NC


def make_in_maps(Zbar, Y, rebalance, gamut):
    Zbar = np.asarray(Zbar, dtype=np.float32)
    Y = np.asarray(Y, dtype=np.float32)
    rebalance = np.asarray(rebalance, dtype=np.float32)
    gamut = np.asarray(gamut, dtype=np.float32)

    gx, gy = gamut[:, 0], gamut[:, 1]
    rhs = np.stack([2.0 * gx, 2.0 * gy, -(gx * gx + gy * gy)]).astype(np.float32)
    rebt = np.ascontiguousarray(
        np.broadcast_to(np.tile(rebalance[:TOPK], TB)[None, :], (P, TB * TOPK))
    ).astype(np.float32)

    in_maps = []
    for c in range(NCORES):
        cs = slice(c * B_PER, (c + 1) * B_PER)
        zc = Zbar[cs].reshape(PIX, Q)
        zb = np.ascontiguousarray(zc.astype(ml_dtypes.bfloat16))
        zq = np.ascontiguousarray(
            zc[:, :TOPK].reshape(NT, P, TOPK).transpose(1, 0, 2).reshape(P, NT * TOPK)
        )
        a = Y[cs, 1].reshape(NT, P)
        b = Y[cs, 2].reshape(NT, P)
        abx = np.zeros((3 * NSEG, ABXC), np.float32)
        for s in range(NSEG):
            abx[3 * s + 0, :PIXSEG] = a[s::NSEG].reshape(-1)
            abx[3 * s + 1, :PIXSEG] = b[s::NSEG].reshape(-1)
            abx[3 * s + 2, :PIXSEG] = 1.0
            abx[3 * s:3 * s + 3, PIXSEG:] = rhs
        in_maps.append({"zbar": zb, "abx": abx, "zq": zq, "rebt": rebt})
    return in_maps


def kernel(Zbar, Y, rebalance, gamut):
    in_maps = make_in_maps(Zbar, Y, rebalance, gamut)
    res = run_bass_kernel_spmd(_get_nc(), in_maps, list(range(NCORES)))
    total = sum(float(r["acc"].sum(dtype=np.float64)) for r in res.results)
    return np.float32(total / (B * H * W))


# revision 27
# speedup vs baseline: 1.5835x; 1.1127x over previous
"""Trainium2 Bass kernel for the colorization loss (v2).

Math (restructured from the reference):
  For each pixel with chroma (a, b):
    m(q)  = 2*a*gx_q + 2*b*gy_q - (gx_q^2 + gy_q^2)   # = (a^2+b^2) - d^2(q)
    top-5 largest m (== 5 smallest distances, ascending), m_0 >= ... >= m_4
    e_k   = exp((m_k - m_0)/50)                        # per-pixel offset cancels
    lse   = log(sum_q exp(Zbar_q))                     # log-softmax denominator
    loss  = mean of (lse * sum_k(reb_k e_k) - sum_k(reb_k e_k Zbar_k)) / sum_k e_k

Key engine assignments (per 128-pixel tile; 256 tiles/core):
  PE   : m matmul (K=3, fp32), 4-way row-tiled via tile_position so
         consecutive tiles run CONCURRENTLY in different 32-row groups.
  DVE  : max8 top-8 of m (the unavoidable 125us), segmented exp-sum
         reduce for the first NGD groups, slim batched epilogue.
  ACT  : exp of zbar. Per-tile exp+accum for most groups (sum on ACT);
         one big group-exp for the first NGD groups (sum on DVE).
         NGD balances ACT vs DVE busy time.
  GPSIMD: Zbar[:, :5] extraction by SBUF copy (replaces descriptor-heavy
         strided DMA), epilogue elementwise muls.
  DMA  : zbar shipped as bf16 (host converts) -> 20.5 MB/core instead of 41.
"""

import numpy as np
import ml_dtypes

import concourse.bass as bass
import concourse.tile as tile
from concourse import mybir
from concourse.bass_utils import run_bass_kernel_spmd

# Problem shape (hardcoded: nn_ColorizationLoss, B,H,W,Q = 16,128,128,313)
B, H, W, Q = 16, 128, 128, 313
NCORES = 8
B_PER = B // NCORES            # 2 images per core
PIX = B_PER * H * W            # 32768 pixels per core
P = 128                        # SBUF partitions / pixels per tile
NT = PIX // P                  # 256 tiles per core
GT = 16                        # tiles per zbar DMA group
NG = NT // GT                  # 16 groups
TB = 64                        # tiles per epilogue batch
NB = NT // TB                  # 4 batches
TOPK = 5
INV50 = 1.0 / 50.0             # 1/(2*sigma^2), sigma=5
NSEG = 4                       # PE row-groups used for the m matmul
PIXSEG = PIX // NSEG           # pixels per abx segment
ABXC = PIXSEG + Q              # abx columns (pixel block + gamut rhs block)
DVE_GROUPS = (1, 5, 9)         # groups whose exp-sum runs on DVE (rest: ACT accum)

f32 = mybir.dt.float32
bf16 = mybir.dt.bfloat16
AF = mybir.ActivationFunctionType
AX = mybir.AxisListType

_NC = None


def _build_nc():
    nc = bass.Bass()
    zbar_d = nc.dram_tensor("zbar", [PIX, Q], f32, kind="ExternalInput")
    # abx packs, per row-group segment s (3 rows each): [a; b; 1] for the
    # pixels of tiles t with t % 4 == s, plus the gamut rhs matrix
    # [2gx; 2gy; -|g|^2] at cols PIXSEG:PIXSEG+Q.
    abx_d = nc.dram_tensor("abx", [3 * NSEG, ABXC], f32, kind="ExternalInput")
    # zbar[:, :5] pre-transposed on host to [P, NT*5] so one DMA with one
    # contiguous run per partition fills Zf (no strided descriptor spam)
    zq_d = nc.dram_tensor("zq", [P, NT * TOPK], f32, kind="ExternalInput")
    reb_d = nc.dram_tensor("rebt", [P, TB * TOPK], f32, kind="ExternalInput")
    out_d = nc.dram_tensor("acc", [P, 1], f32, kind="ExternalOutput")

    # zbar viewed as [group g][partition p][tile-in-group j][q]
    zbar_g = zbar_d[:, :].rearrange("(g j p) q -> g p j q", j=GT, p=P)

    with tile.TileContext(nc) as tc:
        with (
            tc.tile_pool(name="singles", bufs=1) as singles,
            tc.tile_pool(name="zg", bufs=5) as zgp,
            tc.tile_pool(name="esg", bufs=2) as esgp,
            tc.tile_pool(name="est", bufs=2) as estp,
            tc.tile_pool(name="ps", bufs=8, space="PSUM") as psp,
        ):
            abx_sb = singles.tile([32 * (NSEG - 1) + 3, ABXC], f32)
            for s in range(NSEG):
                nc.sync.dma_start(
                    out=abx_sb[32 * s:32 * s + 3, :], in_=abx_d[3 * s:3 * s + 3, :]
                )
            # stage rebalance through an ACT copy: the gpsimd U-mul then waits
            # on the single Activation sem, which covers BOTH its E input and
            # reb (one legal sem wait instead of two)
            reb_st = singles.tile([P, TB, TOPK], f32)
            nc.sync.dma_start(
                out=reb_st, in_=reb_d[:, :].rearrange("p (t k) -> p t k", k=TOPK)
            )
            reb_sb = singles.tile([P, TB, TOPK], f32)
            nc.scalar.copy(reb_sb, reb_st)
            acc = singles.tile([P, 1], f32)
            nc.vector.memset(acc, 0.0)

            # Full-size buffers (one column range per tile/batch, never
            # recycled) — no WAR/WAW slot hazards, so every cross-engine
            # dependency costs at most the single legal sem wait.
            Sf = singles.tile([P, NT], f32)          # sum_q exp(zbar)
            Wf = singles.tile([P, NT, 8], f32)       # top-8 of m
            Zf = singles.tile([P, NT, TOPK], f32)    # zbar[:, :5]
            # bump scratch (see epilogue)
            bmpa = singles.tile([P, NB], f32)
            bmpv = singles.tile([P, NB], f32)
            bmpp = singles.tile([P, NB], f32)
            Xf = singles.tile([P, NT, TOPK], f32)    # m_k - m_0 (then U*z)
            Ef = singles.tile([P, NT, TOPK], f32)    # exp(X/50)
            Uf = singles.tile([P, NT, TOPK], f32)    # E * reb
            s2f = singles.tile([P, NT], f32)
            s1f = singles.tile([P, NT], f32)
            swf = singles.tile([P, NT], f32)
            lsef = singles.tile([P, NT], f32)
            t1f = singles.tile([P, NT], f32)
            rf = singles.tile([P, NT], f32)
            ppf = singles.tile([P, NB], f32)

            zg_pending = {}

            # zbar group triggers on the ACT ring: ACT is a reader of zg, so
            # the slot's ACT WAR is covered by sequencer program order and the
            # trigger carries only the gpsimd (Zf copy) wait.
            def issue_zg(g):
                zgt = zgp.tile([P, GT, Q], f32, tag="zg", name=f"zg{g}")
                nc.scalar.dma_start(out=zgt, in_=zbar_g[g])
                zg_pending[g] = zgt

            issue_zg(0)
            issue_zg(1)

            for g in range(NG):
                if g + 2 < NG:
                    issue_zg(g + 2)
                if g == 2:
                    # Zf load deferred until here (ACT ring, so it fires
                    # once ACT reaches this point): keeps the first zbar
                    # groups' DMA bandwidth unshared during warmup. Not
                    # needed before the batch-0 epilogue (after group 3).
                    nc.scalar.dma_start(
                        out=Zf,
                        in_=zq_d[:, :].rearrange("p (t k) -> p t k", k=TOPK),
                    )
                    # gpsimd warmup touch of Zf: absorbs the Zf-DMA wait
                    # into Pool program order so the UZ muls carry no DMA wait
                    zf_tch = singles.tile([P, 1], f32)
                    nc.gpsimd.tensor_copy(zf_tch, Zf[:, 0, 0:1])
                zg = zg_pending.pop(g)

                for j in range(GT):
                    t = g * GT + j
                    s = t % NSEG                     # PE row-group
                    blk = t // NSEG                  # column block in segment
                    ps = psp.tile([P, Q], f32, tag="ps")
                    nc.tensor.matmul(
                        ps,
                        abx_sb[32 * s:32 * s + 3, blk * P:(blk + 1) * P],
                        abx_sb[32 * s:32 * s + 3, PIXSEG:PIXSEG + Q],
                        start=True,
                        stop=True,
                        tile_position=(32 * s, 0),
                    )
                    nc.vector.max(out=Wf[:, t, :], in_=ps)

                if g in DVE_GROUPS:
                    # group-sized exp on ACT; per-tile sums on DVE
                    esg = esgp.tile([P, GT, Q], bf16, tag="esg")
                    nc.scalar.activation(out=esg, in_=zg[:, :, :], func=AF.Exp)
                    nc.vector.reduce_sum(
                        Sf[:, g * GT:(g + 1) * GT], esg, axis=AX.X
                    )
                else:
                    # per-tile exp with ACT-side accumulation. The group
                    # boundary (j == 0) uses its own scratch tag: its WAW dep
                    # is then ancient, so it carries only the zbar-DMA wait
                    # (compute instructions allow a single sem wait).
                    for j in range(GT):
                        t = g * GT + j
                        es = estp.tile([P, Q], bf16, tag="esb" if j == 0 else "est")
                        nc.scalar.activation(
                            out=es, in_=zg[:, j, :], func=AF.Exp,
                            accum_out=Sf[:, t:t + 1],
                        )

                # ---- batched epilogue every TB tiles ----
                if (g + 1) % (TB // GT) == 0:
                    bi = (g + 1) // (TB // GT) - 1
                    sl = slice(bi * TB, (bi + 1) * TB)
                    Xt = Xf[:, sl]
                    nc.vector.tensor_sub(
                        Xt, Wf[:, sl, 0:TOPK],
                        Wf[:, sl, 0:1].broadcast_to([P, TB, TOPK]),
                    )
                    E = Ef[:, sl]
                    nc.scalar.activation(out=E, in_=Xt, func=AF.Exp, scale=INV50)
                    U = Uf[:, sl]
                    nc.gpsimd.tensor_mul(U, E, reb_sb)
                    # UZ overwrites Xf's batch range (E already consumed it).
                    # gpsimd bump: absorb the DVE WAW (the sub that wrote Xf)
                    # so the UZ mul itself runs on Pool program order.
                    nc.gpsimd.tensor_copy(bmpp[:, bi:bi + 1], Xf[:, bi * TB, 0:1])
                    UZ = Xf[:, sl]
                    nc.gpsimd.tensor_mul(UZ, U, Zf[:, sl, :])
                    s2 = s2f[:, sl]
                    nc.vector.reduce_sum(s2, U, axis=AX.X)
                    s1 = s1f[:, sl]
                    nc.vector.reduce_sum(s1, UZ, axis=AX.X)
                    sw = swf[:, sl]
                    nc.vector.reduce_sum(sw, E, axis=AX.X)
                    # ACT bump: absorb the DVE dep (this batch's DVE-group
                    # reduce into Sf) so lse itself carries one wait
                    gd = [x for x in DVE_GROUPS if bi * (TB // GT) <= x < (bi + 1) * (TB // GT)]
                    if gd:
                        nc.scalar.copy(bmpa[:, bi:bi + 1], Sf[:, gd[0] * GT:gd[0] * GT + 1])
                    lse = lsef[:, sl]
                    nc.scalar.activation(out=lse, in_=Sf[:, sl], func=AF.Ln)
                    # DVE bump: absorb the ACT lse dep so the t1 chain stays
                    # on DVE program order
                    nc.vector.tensor_copy(bmpv[:, bi:bi + 1], lse[:, 0:1])
                    t1 = t1f[:, sl]
                    nc.vector.tensor_mul(t1, lse, s2)
                    nc.vector.tensor_sub(t1, t1, s1)
                    r = rf[:, sl]
                    nc.vector.reciprocal(r, sw)
                    nc.vector.tensor_mul(t1, t1, r)
                    nc.vector.reduce_sum(ppf[:, bi:bi + 1], t1, axis=AX.X)
                    nc.vector.tensor_add(acc, acc, ppf[:, bi:bi + 1])

            # SWDGE (gpsimd) ring: fresh sem pool for the final transfer
            nc.gpsimd.dma_start(out=out_d[:, :], in_=acc)

    # The kernel-tail drain waits on every used proc which can exceed the
    # instruction's sync-wait capacity. Every instruction is transitively
    # upstream of the final out DMA (acc is the sink), so waiting for that
    # DMA's SWDGE sem alone is sufficient.
    for blk in nc.m.functions[0].blocks:
        for inst in blk.instructions:
            si = getattr(inst, "sync_info", None)
            if si is None or type(inst).__name__ != "InstDrain":
                continue
            ge = [w for w in si.on_wait if w.wait_mode == "sem-ge-imm"]
            if len(ge) >= 2:
                sw_ = [w for w in ge if "DMASW" in w.ant_name]
                assert sw_, f"tail drain has no SWDGE wait: {ge}"
                si.on_wait = sw_[:1]
    return nc


def _get_nc():
    global _NC
    if _NC is None:
        _NC = _build_nc()
    return _NC


def make_in_maps(Zbar, Y, rebalance, gamut):
    Zbar = np.asarray(Zbar, dtype=np.float32)
    Y = np.asarray(Y, dtype=np.float32)
    rebalance = np.asarray(rebalance, dtype=np.float32)
    gamut = np.asarray(gamut, dtype=np.float32)

    gx, gy = gamut[:, 0], gamut[:, 1]
    rhs = np.stack([2.0 * gx, 2.0 * gy, -(gx * gx + gy * gy)]).astype(np.float32)
    rebt = np.ascontiguousarray(
        np.broadcast_to(np.tile(rebalance[:TOPK], TB)[None, :], (P, TB * TOPK))
    ).astype(np.float32)

    in_maps = []
    for c in range(NCORES):
        cs = slice(c * B_PER, (c + 1) * B_PER)
        zc = Zbar[cs].reshape(PIX, Q)
        zb = np.ascontiguousarray(zc)
        zq = np.ascontiguousarray(
            zc[:, :TOPK].reshape(NT, P, TOPK).transpose(1, 0, 2).reshape(P, NT * TOPK)
        )
        a = Y[cs, 1].reshape(NT, P)
        b = Y[cs, 2].reshape(NT, P)
        abx = np.zeros((3 * NSEG, ABXC), np.float32)
        for s in range(NSEG):
            abx[3 * s + 0, :PIXSEG] = a[s::NSEG].reshape(-1)
            abx[3 * s + 1, :PIXSEG] = b[s::NSEG].reshape(-1)
            abx[3 * s + 2, :PIXSEG] = 1.0
            abx[3 * s:3 * s + 3, PIXSEG:] = rhs
        in_maps.append({"zbar": zb, "abx": abx, "zq": zq, "rebt": rebt})
    return in_maps


def kernel(Zbar, Y, rebalance, gamut):
    in_maps = make_in_maps(Zbar, Y, rebalance, gamut)
    res = run_bass_kernel_spmd(_get_nc(), in_maps, list(range(NCORES)))
    total = sum(float(r["acc"].sum(dtype=np.float64)) for r in res.results)
    return np.float32(total / (B * H * W))
